# revision 8
# baseline (speedup 1.0000x reference)
"""BipartiteGATConv on 8 Trainium2 NeuronCores (Bass/Tile).

Strategy (dst-sharded, zero collectives):
- dst nodes partitioned across 8 cores (6250 rows each); host routes edges to
  the core owning their dst, groups them by 128-row dst block, splits each
  block's edges into lo/hi src halves (dma_gather int16 index limit), pads to
  128-edge tiles with a schedule that is identical across cores (SPMD).
- Phase 0 (on device, replicated): src table [50048, 128] bf16 rows = x_src @
  W_src, built by direct matmul from host-transposed bf16 x_srcT (no on-chip
  transposes). Dst-side self term (x_dst@W_self + b) and per-dst alpha_dst
  kept SBUF-resident.
- Edge phase: per 8-tile batch, dma_gather pulls 128 bf16 feats per edge
  (256B rows). alpha_src is computed on chip: feats * att (Pool) then a
  grouped 32-wide reduce (DVE). alpha_dst is expanded per-edge via a
  transposed one-hot (is_equal vs a broadcast-DMA'd [1,1024] row) contracted
  with the resident alpha_dst block on the PE. p = exp(max(u, 0.2u)).
  Messages = feats * p, aggregated per dst block by one-hot-stationary
  matmuls accumulating in PSUM; the softmax denominator accumulates from a
  strided view of p (4 extra PSUM columns). Normalize + self-term at block
  finalize.
"""
import math
import numpy as np
import ml_dtypes

import concourse.bass as bass
import concourse.bacc as bacc
import concourse.tile as tile
from concourse import mybir
from concourse.bass_utils import run_bass_kernel_spmd

N_SRC = 50000
N_DST = 50000
OUT_DIM = 128
HEADS = 4
D_HEAD = 32
NCORES = 8
DST_PER_CORE = N_DST // NCORES          # 6250
BLK = 128
N_BLK = math.ceil(DST_PER_CORE / BLK)   # 49
DST_PAD = N_BLK * BLK                   # 6272
HALF = 32768                            # lo/hi src split for int16 gather idx
SRC_PAD = 391 * 128                     # 50048 table rows
ROW = 128                               # bf16 elems per table row (256B)
P = 128
BATCH = 8                               # tiles per batch
BF = mybir.dt.bfloat16
F32 = mybir.dt.float32


def _wrap16(idx_i16):
    """[n] -> [128, n//16] int16 wrapped in 16 partitions, replicated x8."""
    n = idx_i16.shape[0]
    w = idx_i16.reshape(n // 16, 16).T  # [16, n/16]
    return np.tile(w, (8, 1))


def _preprocess(edge_src, edge_dst):
    """Route edges, build per-core tile streams + shared schedule."""
    es = np.asarray(edge_src).astype(np.int64)
    ed = np.asarray(edge_dst).astype(np.int64)
    core = ed // DST_PER_CORE
    shard = ed % DST_PER_CORE
    blk = shard // BLK
    edl = shard % BLK

    # per (core, blk, half) edge index lists
    counts = np.zeros((NCORES, N_BLK, 2), dtype=np.int64)
    lists = [[[None, None] for _ in range(N_BLK)] for _ in range(NCORES)]
    half = (es >= HALF).astype(np.int64)
    order = np.lexsort((half, blk, core))
    es_s, blk_s, edl_s, core_s, half_s = (
        es[order], blk[order], edl[order], core[order], half[order])
    key = ((core_s * N_BLK) + blk_s) * 2 + half_s
    uniq, starts = np.unique(key, return_index=True)
    starts = list(starts) + [len(key)]
    for i, k in enumerate(uniq):
        c = int(k) // (N_BLK * 2)
        b = (int(k) // 2) % N_BLK
        h = int(k) % 2
        sl = slice(starts[i], starts[i + 1])
        lists[c][b][h] = (es_s[sl], edl_s[sl])
        counts[c, b, h] = starts[i + 1] - starts[i]

    # shared tile schedule: per block, lo tiles then hi tiles (max over cores)
    t_lo = np.maximum(1, np.ceil(counts[:, :, 0] / P).astype(np.int64).max(axis=0))
    t_hi = np.ceil(counts[:, :, 1] / P).astype(np.int64).max(axis=0)
    tiles = []  # (blk, half, first_in_blk, last_in_blk)
    for b in range(N_BLK):
        n = int(t_lo[b] + t_hi[b])
        for j in range(int(t_lo[b])):
            tiles.append((b, 0, j == 0, j == n - 1))
        for j in range(int(t_hi[b])):
            tiles.append((b, 1, t_lo[b] == 0 and j == 0, j == int(t_hi[b]) - 1))
    T = len(tiles)
    while T % BATCH != 0:
        tiles.append((N_BLK - 1, 0, False, False))
        T += 1
    # padded trailing tiles fold into block N_BLK-1's accumulation
    lastb = N_BLK - 1
    idxs = [i for i, t in enumerate(tiles) if t[0] == lastb]
    for j, i in enumerate(idxs):
        b, h, _, _ = tiles[i]
        tiles[i] = (b, h, j == 0, j == len(idxs) - 1)

    NB = T // BATCH

    # per-batch gather runs: (half, off_tiles, ntiles) split at batch bounds
    runs = []
    for bi in range(NB):
        rr = []
        t0 = bi * BATCH
        cur_h, cur_off = tiles[t0][1], 0
        for j in range(1, BATCH):
            h = tiles[t0 + j][1]
            if h != cur_h:
                rr.append((cur_h, cur_off, j - cur_off))
                cur_h, cur_off = h, j
        rr.append((cur_h, cur_off, BATCH - cur_off))
        runs.append(rr)

    # per-core arrays
    per_core = []
    for c in range(NCORES):
        es_tiles = np.zeros((T, P), dtype=np.int64)
        edl_tiles = np.full((T, P), -1.0, dtype=np.float32)
        fill = np.zeros(N_BLK * 2, dtype=np.int64)
        for ti, (b, h, _, _) in enumerate(tiles):
            ent = lists[c][b][h]
            if ent is None:
                if h == 1:
                    es_tiles[ti, :] = HALF
                continue
            e_arr, l_arr = ent
            k = fill[b * 2 + h]
            take = min(P, max(0, len(e_arr) - k))
            if take > 0:
                es_tiles[ti, :take] = e_arr[k:k + take]
                edl_tiles[ti, :take] = l_arr[k:k + take]
            if take < P and h == 1:
                es_tiles[ti, take:] = HALF
            fill[b * 2 + h] = k + take
        loc = es_tiles.copy()
        for ti, (b, h, _, _) in enumerate(tiles):
            if h == 1:
                loc[ti] -= HALF
        esw = np.zeros((NB, P, BATCH * P // 16), dtype=np.int16)
        for bi in range(NB):
            flat = loc[bi * BATCH:(bi + 1) * BATCH].reshape(-1).astype(np.int16)
            esw[bi] = _wrap16(flat)
        el_b = edl_tiles.reshape(NB, BATCH, P).transpose(0, 2, 1)  # [NB,P,8]
        edl1 = edl_tiles.reshape(NB, BATCH * P)                    # [NB,1024]
        per_core.append({
            "esw": esw,
            "el": np.ascontiguousarray(el_b).astype(ml_dtypes.bfloat16),
            "edl1": np.ascontiguousarray(edl1).astype(ml_dtypes.bfloat16),
        })
    return tiles, runs, NB, per_core


def _build(tiles, runs, NB):
    nc = bacc.Bacc("TRN2", target_bir_lowering=False, debug=False,
                   enable_asserts=True, num_devices=NCORES,
                   num_swdge_queues=4)

    xsT = nc.dram_tensor("xsT", [128, SRC_PAD], BF, kind="ExternalInput")
    xdT = nc.dram_tensor("xdT", [128, DST_PAD], F32, kind="ExternalInput")
    wsrc = nc.dram_tensor("wsrc", [128, 128], BF, kind="ExternalInput")
    rhs_d = nc.dram_tensor("rhs_d", [128, 132], F32, kind="ExternalInput")
    b_row = nc.dram_tensor("b_row", [1, 128], F32, kind="ExternalInput")
    att8 = nc.dram_tensor("att8", [1, BATCH * P], BF, kind="ExternalInput")
    esw = nc.dram_tensor("esw", [NB, P, BATCH * P // 16], mybir.dt.int16,
                         kind="ExternalInput")
    el = nc.dram_tensor("el", [NB, P, BATCH], BF, kind="ExternalInput")
    edl1 = nc.dram_tensor("edl1", [NB, BATCH * P], BF, kind="ExternalInput")
    kin = nc.dram_tensor("kin", [1, 1], mybir.dt.int32, kind="ExternalInput")
    y = nc.dram_tensor("y", [DST_PAD, 128], F32, kind="ExternalOutput")

    table_lo = nc.dram_tensor("table_lo", [HALF, ROW], BF)
    table_hi = nc.dram_tensor("table_hi", [SRC_PAD - HALF, ROW], BF)

    with tile.TileContext(nc) as tc:
        with tc.tile_pool(name="const", bufs=1) as cpool, \
             tc.tile_pool(name="resident", bufs=1) as rpool:
            kt = cpool.tile([1, 1], mybir.dt.int32)
            nc.sync.dma_start(out=kt[:], in_=kin[:, :])
            kv = nc.values_load(kt[0:1, 0:1], min_val=0, max_val=100000,
                                skip_runtime_bounds_check=True)
            iota_i = cpool.tile([P, P], mybir.dt.int32)
            nc.gpsimd.iota(iota_i[:], pattern=[[1, P]], base=0,
                           channel_multiplier=0)
            iota_rep = cpool.tile([P, P], BF)
            nc.vector.tensor_copy(iota_rep[:], iota_i[:])
            iotap_i = cpool.tile([P, BATCH * P], mybir.dt.int32)
            nc.gpsimd.iota(iotap_i[:], pattern=[[0, BATCH * P]], base=0,
                           channel_multiplier=1)
            iotap = cpool.tile([P, BATCH * P], BF)
            nc.vector.tensor_copy(iotap[:], iotap_i[:])
            ones1 = cpool.tile([1, P], F32)
            nc.vector.memset(ones1[:], 1.0)
            wsrc_t = cpool.tile([P, 128], BF)
            nc.sync.dma_start(out=wsrc_t[:], in_=wsrc[:, :])
            rhsd_t = cpool.tile([P, 132], F32)
            nc.sync.dma_start(out=rhsd_t[:], in_=rhs_d[:, :])
            b_t = cpool.tile([1, P], F32)
            nc.sync.dma_start(out=b_t[:], in_=b_row[:, :])
            attrep = cpool.tile([P, BATCH * P], BF)
            nc.sync.dma_start(out=attrep[:],
                              in_=att8[0:1, :].to_broadcast([P, BATCH * P]))

            self_all = rpool.tile([P, N_BLK * 128], F32)
            alpha_d = rpool.tile([P, N_BLK * 4], BF)

            with tc.For_i(0, kv) as _i:
                # ---------------- phase 0: src table ----------------
                GRP = 8
                nsrc_blk = SRC_PAD // P                      # 391
                ngrp = math.ceil(nsrc_blk / GRP)
                with tc.tile_pool(name="p1x", bufs=3) as xpool, \
                     tc.tile_pool(name="p1f", bufs=3) as fpool, \
                     tc.tile_pool(name="p1ps", bufs=4, space="PSUM") as psum:
                    for g in range(ngrp):
                        j0 = g * GRP
                        jn = min(GRP, nsrc_blk - j0)
                        xg = xpool.tile([P, GRP * 128], BF, tag="xg")
                        nc.sync.dma_start(
                            out=xg[:, :jn * 128],
                            in_=xsT[:, j0 * 128:(j0 + jn) * 128])
                        fb = fpool.tile([P, GRP * ROW], BF, tag="fb")
                        for j4 in range(jn):
                            jj = j0 + j4
                            ps = psum.tile([P, 128], F32, tag="ps")
                            nc.tensor.matmul(
                                ps[:], lhsT=xg[:, j4 * 128:(j4 + 1) * 128],
                                rhs=wsrc_t[:], start=True, stop=True)
                            dst_sl = fb[:, j4 * ROW:(j4 + 1) * ROW]
                            if jj % 2 == 0:
                                nc.vector.tensor_copy(dst_sl, ps[:])
                            else:
                                nc.scalar.copy(dst_sl, ps[:])
                        r0 = j0 * P
                        fb3 = fb[:].rearrange("p (j c) -> p j c", c=ROW)
                        segs = []
                        if r0 < HALF:
                            nlo = min(jn, (HALF - r0) // P)
                            segs.append((table_lo, r0, 0, nlo))
                            if nlo < jn:
                                segs.append((table_hi, 0, nlo, jn - nlo))
                        else:
                            segs.append((table_hi, r0 - HALF, 0, jn))
                        for (tdst, rr, joff, jcnt) in segs:
                            nc.sync.dma_start(
                                out=tdst[rr:rr + jcnt * P, :].rearrange(
                                    "(j p) c -> p j c", p=P),
                                in_=fb3[:, joff:joff + jcnt, :])

                # ---------------- phase 0: dst side ----------------
                with tc.tile_pool(name="p0", bufs=3) as pool, \
                     tc.tile_pool(name="p0ps", bufs=3, space="PSUM") as psum, \
                     tc.tile_pool(name="p0ps2", bufs=2, space="PSUM") as psum2:
                    DG = 8
                    for g0 in range(0, N_BLK, DG):
                        gn = min(DG, N_BLK - g0)
                        xdg = pool.tile([P, DG * 128], F32, tag="xdg")
                        nc.sync.dma_start(
                            out=xdg[:, :gn * 128],
                            in_=xdT[:, g0 * 128:(g0 + gn) * 128])
                        for jo in range(gn):
                            j = g0 + jo
                            xsl = xdg[:, jo * 128:(jo + 1) * 128]
                            ps_s = psum2.tile([P, P], F32, tag="ps_s")
                            nc.tensor.matmul(ps_s[:], lhsT=ones1[:],
                                             rhs=b_t[:],
                                             start=True, stop=False)
                            nc.tensor.matmul(ps_s[:], lhsT=xsl,
                                             rhs=rhsd_t[:, 0:128],
                                             start=False, stop=True)
                            ps_a = psum.tile([P, 4], F32, tag="ps_a")
                            nc.tensor.matmul(ps_a[:], lhsT=xsl,
                                             rhs=rhsd_t[:, 128:132],
                                             start=True, stop=True)
                            if j % 2 == 0:
                                nc.scalar.copy(
                                    self_all[:, j * 128:(j + 1) * 128],
                                    ps_s[:])
                            else:
                                nc.vector.tensor_copy(
                                    self_all[:, j * 128:(j + 1) * 128],
                                    ps_s[:])
                            nc.vector.tensor_copy(
                                alpha_d[:, j * 4:(j + 1) * 4], ps_a[:])

                # ---------------- edge phase ----------------
                with tc.tile_pool(name="eg", bufs=6) as gpool, \
                     tc.tile_pool(name="ei", bufs=6) as ipool, \
                     tc.tile_pool(name="er", bufs=6) as erpool, \
                     tc.tile_pool(name="es", bufs=6) as spool, \
                     tc.tile_pool(name="ea", bufs=4) as apool, \
                     tc.tile_pool(name="em", bufs=6) as mpool, \
                     tc.tile_pool(name="eu", bufs=6) as upool, \
                     tc.tile_pool(name="eo", bufs=4) as opool, \
                     tc.tile_pool(name="eps", bufs=4, space="PSUM") as psA, \
                     tc.tile_pool(name="eac", bufs=3, space="PSUM") as psB:
                    qn = 0
                    acc = None
                    for bi in range(NB):
                        it = ipool.tile([P, BATCH * P // 16], mybir.dt.int16,
                                        tag="it")
                        nc.sync.dma_start(out=it[:], in_=esw[bi, :, :])
                        elb = ipool.tile([P, BATCH], BF, tag="elb")
                        nc.sync.dma_start(out=elb[:], in_=el[bi, :, :])
                        er1 = erpool.tile([1, BATCH * P], BF, tag="er1")
                        nc.scalar.dma_start(out=er1[:],
                                            in_=edl1[bi:bi + 1, :])
                        er = erpool.tile([P, BATCH * P], BF, tag="er")
                        nc.gpsimd.partition_broadcast(er[:], er1[:],
                                                      channels=P)
                        g8 = gpool.tile([P, BATCH * ROW], BF, tag="g8")
                        g83 = g8[:].rearrange("p (t c) -> p t c", c=ROW)
                        for (h, off, ntl) in runs[bi]:
                            tsrc = table_hi if h == 1 else table_lo
                            nc.gpsimd.dma_gather(
                                out_ap=g83[:, off:off + ntl, :],
                                in_ap=tsrc[:, :],
                                idxs_ap=it[:, off * 8:(off + ntl) * 8],
                                num_idxs=ntl * P,
                                num_idxs_reg=ntl * P,
                                elem_size=ROW,
                                single_packet=False,
                                queue_num=qn % 4,
                            )
                            qn += 1
                        s8 = spool.tile([P, BATCH * P], BF, tag="s8")
                        nc.vector.tensor_tensor(
                            out=s8[:].rearrange("p (t r) -> p t r", r=P),
                            in0=elb[:, :, None].to_broadcast([P, BATCH, P]),
                            in1=iota_rep[:, None, :].to_broadcast(
                                [P, BATCH, P]),
                            op=mybir.AluOpType.is_equal)
                        st8 = spool.tile([P, BATCH * P], BF, tag="st8")
                        nc.vector.tensor_tensor(
                            out=st8[:], in0=iotap[:], in1=er[:],
                            op=mybir.AluOpType.is_equal)
                        amul = apool.tile([P, BATCH * P], BF, tag="amul")
                        nc.gpsimd.tensor_tensor(
                            out=amul[:], in0=g8[:], in1=attrep[:],
                            op=mybir.AluOpType.mult)
                        av4 = upool.tile([P, BATCH * 4], F32, tag="av4")
                        nc.vector.tensor_reduce(
                            av4[:].rearrange("p (g o) -> p g o", o=1),
                            amul[:].rearrange("p (g d) -> p g d", d=D_HEAD),
                            mybir.AxisListType.X, mybir.AluOpType.add)
                        ade = psA.tile([P, BATCH * 4], F32, tag="ade")
                        for t in range(BATCH):
                            blk = tiles[bi * BATCH + t][0]
                            nc.tensor.matmul(
                                ade[:, t * 4:(t + 1) * 4],
                                lhsT=st8[:, t * P:(t + 1) * P],
                                rhs=alpha_d[:, blk * 4:(blk + 1) * 4],
                                start=True, stop=True)
                        u8 = upool.tile([P, BATCH * 4], F32, tag="u8")
                        nc.vector.tensor_tensor(
                            out=u8[:], in0=av4[:], in1=ade[:],
                            op=mybir.AluOpType.add)
                        lk = upool.tile([P, BATCH * 4], F32, tag="lk")
                        nc.vector.scalar_tensor_tensor(
                            out=lk[:], in0=u8[:], scalar=0.2, in1=u8[:],
                            op0=mybir.AluOpType.mult, op1=mybir.AluOpType.max)
                        pexp = mpool.tile([P, BATCH * P], BF, tag="pexp")
                        pexp4 = pexp[:].rearrange(
                            "p (t h d) -> p t h d", h=HEADS, d=D_HEAD)
                        lk3 = lk[:].rearrange("p (t h) -> p t h", h=HEADS)
                        nc.scalar.activation(
                            pexp4, lk3[:, :, :, None].to_broadcast(
                                [P, BATCH, HEADS, D_HEAD]),
                            mybir.ActivationFunctionType.Exp)
                        mp8 = mpool.tile([P, BATCH * 132], BF, tag="mp8")
                        mp83 = mp8[:].rearrange("p (t c) -> p t c", c=132)
                        HT = BATCH // 2
                        nc.vector.tensor_tensor(
                            out=mp83[:, 0:HT, 0:128],
                            in0=g83[:, 0:HT, :],
                            in1=pexp[:].rearrange("p (t c) -> p t c",
                                                  c=P)[:, 0:HT, :],
                            op=mybir.AluOpType.mult)
                        nc.gpsimd.tensor_tensor(
                            out=mp83[:, HT:, 0:128],
                            in0=g83[:, HT:, :],
                            in1=pexp[:].rearrange("p (t c) -> p t c",
                                                  c=P)[:, HT:, :],
                            op=mybir.AluOpType.mult)
                        nc.scalar.copy(
                            mp83[:, :, 128:132],
                            pexp4[:, :, :, 0])
                        for t in range(BATCH):
                            ti = bi * BATCH + t
                            blk, _, first, last = tiles[ti]
                            if first:
                                acc = psB.tile([P, 132], F32, tag="acc")
                            s8t = s8[:, t * P:(t + 1) * P]
                            nc.tensor.matmul(
                                acc[:], lhsT=s8t,
                                rhs=mp83[:, t, :],
                                start=first, stop=last)
                            if last:
                                s1 = upool.tile([P, 4], F32, tag="s1")
                                nc.vector.tensor_scalar_add(
                                    s1[:], acc[:, 128:132], 1e-16)
                                rv = upool.tile([P, 4], F32, tag="rv")
                                nc.vector.reciprocal(rv[:], s1[:])
                                ob = opool.tile([P, P], F32, tag="ob")
                                nc.vector.tensor_tensor(
                                    out=ob[:].rearrange(
                                        "p (h d) -> p h d", h=HEADS),
                                    in0=acc[:, 0:128].rearrange(
                                        "p (h d) -> p h d", h=HEADS),
                                    in1=rv[:, :, None].to_broadcast(
                                        [P, HEADS, D_HEAD]),
                                    op=mybir.AluOpType.mult)
                                ob2 = opool.tile([P, P], F32, tag="ob2")
                                nc.gpsimd.tensor_tensor(
                                    out=ob2[:], in0=ob[:],
                                    in1=self_all[:,
                                                 blk * 128:(blk + 1) * 128],
                                    op=mybir.AluOpType.add)
                                nc.scalar.dma_start(
                                    out=y[blk * P:(blk + 1) * P, :],
                                    in_=ob2[:])
    nc.compile()
    return nc


def _host_arrays(x_src, x_dst, W_src, W_dst, att_src, att_dst, W_self, b_self):
    x_src = np.asarray(x_src, dtype=np.float32)
    x_dst = np.asarray(x_dst, dtype=np.float32)
    W_src = np.asarray(W_src, dtype=np.float32)
    W_dst = np.asarray(W_dst, dtype=np.float32)
    att_src = np.asarray(att_src, dtype=np.float32).reshape(HEADS, D_HEAD)
    att_dst = np.asarray(att_dst, dtype=np.float32).reshape(HEADS, D_HEAD)
    W_self = np.asarray(W_self, dtype=np.float32)
    b_self = np.asarray(b_self, dtype=np.float32)

    A_d = np.zeros((128, HEADS), dtype=np.float32)
    for h in range(HEADS):
        A_d[h * D_HEAD:(h + 1) * D_HEAD, h] = att_dst[h]
    WA_d = W_dst @ A_d                                      # [128, 4]
    rhs_d = np.concatenate([W_self, WA_d], axis=1)          # [128, 132] f32
    x_src_p = np.zeros((SRC_PAD, 128), dtype=np.float32)
    x_src_p[:N_SRC] = x_src
    xsT = np.ascontiguousarray(x_src_p.T).astype(ml_dtypes.bfloat16)
    wsrc_bf = W_src.astype(ml_dtypes.bfloat16)
    att8 = np.tile(att_src.reshape(1, -1), (1, BATCH)).astype(
        ml_dtypes.bfloat16)                                 # [1, 1024]
    return xsT, x_dst, wsrc_bf, rhs_d, b_self.reshape(1, 128), att8


_CACHE = {}


def _get_program(edge_src, edge_dst):
    key = (hash(np.asarray(edge_src).tobytes()),
           hash(np.asarray(edge_dst).tobytes()))
    if key not in _CACHE:
        tiles, runs, NB, per_core = _preprocess(edge_src, edge_dst)
        nc = _build(tiles, runs, NB)
        _CACHE[key] = (nc, per_core)
    return _CACHE[key]


def kernel(x_src, x_dst, edge_src, edge_dst, num_dst,
           W_src, W_dst, att_src, att_dst, W_self, b_self, _k=1):
    nc, per_core = _get_program(edge_src, edge_dst)
    xsT, x_dst_f, wsrc_bf, rhs_d, b_row, att8 = _host_arrays(
        x_src, x_dst, W_src, W_dst, att_src, att_dst, W_self, b_self)

    in_maps = []
    for c in range(NCORES):
        xd = np.zeros((DST_PAD, 128), dtype=np.float32)
        xd[:DST_PER_CORE] = x_dst_f[c * DST_PER_CORE:(c + 1) * DST_PER_CORE]
        in_maps.append({
            "xsT": xsT,
            "xdT": np.ascontiguousarray(xd.T),
            "wsrc": wsrc_bf,
            "rhs_d": rhs_d,
            "b_row": b_row,
            "att8": att8,
            "esw": per_core[c]["esw"],
            "el": per_core[c]["el"],
            "edl1": per_core[c]["edl1"],
            "kin": np.array([[_k]], dtype=np.int32),
        })
    res = run_bass_kernel_spmd(nc, in_maps, list(range(NCORES)))
    out = np.concatenate(
        [res.results[c]["y"][:DST_PER_CORE] for c in range(NCORES)], axis=0)
    return out.astype(np.float32)


# revision 9
# speedup vs baseline: 2.3607x; 2.3607x over previous
"""BipartiteGATConv on 8 Trainium2 NeuronCores (Bass/Tile).

Strategy (dst-sharded, zero collectives):
- dst nodes partitioned across 8 cores (6250 rows each); host routes edges to
  the core owning their dst, groups them by 128-row dst block, splits each
  block's edges into lo/hi src halves (dma_gather int16 index limit), pads to
  128-edge tiles with a schedule that is identical across cores (SPMD).
- Phase 0 (on device, replicated): src table [50048, 128] bf16 rows = x_src @
  W_src, built by direct matmul from host-transposed bf16 x_srcT (no on-chip
  transposes). Dst-side self term (x_dst@W_self + b) and per-dst alpha_dst
  kept SBUF-resident.
- Edge phase: per 8-tile batch, dma_gather pulls 128 bf16 feats per edge
  (256B rows). alpha_src is computed on chip: feats * att (Pool) then a
  grouped 32-wide reduce (DVE). alpha_dst is expanded per-edge via a
  transposed one-hot (is_equal vs a broadcast-DMA'd [1,1024] row) contracted
  with the resident alpha_dst block on the PE. p = exp(max(u, 0.2u)).
  Messages = feats * p, aggregated per dst block by one-hot-stationary
  matmuls accumulating in PSUM; the softmax denominator accumulates from a
  strided view of p (4 extra PSUM columns). Normalize + self-term at block
  finalize.
"""
import math
import numpy as np
import ml_dtypes

import concourse.bass as bass
import concourse.bacc as bacc
import concourse.tile as tile
from concourse import mybir
from concourse.bass_utils import run_bass_kernel_spmd

N_SRC = 50000
N_DST = 50000
OUT_DIM = 128
HEADS = 4
D_HEAD = 32
NCORES = 8
DST_PER_CORE = N_DST // NCORES          # 6250
BLK = 128
N_BLK = math.ceil(DST_PER_CORE / BLK)   # 49
DST_PAD = N_BLK * BLK                   # 6272
HALF = 32768                            # lo/hi src split for int16 gather idx
SRC_PAD = 391 * 128                     # 50048 table rows
ROW = 128                               # bf16 elems per table row (256B)
P = 128
BATCH = 8                               # tiles per batch
BF = mybir.dt.bfloat16
F32 = mybir.dt.float32


def _wrap16(idx_i16):
    """[n] -> [128, n//16] int16 wrapped in 16 partitions, replicated x8."""
    n = idx_i16.shape[0]
    w = idx_i16.reshape(n // 16, 16).T  # [16, n/16]
    return np.tile(w, (8, 1))


def _preprocess(edge_src, edge_dst):
    """Route edges, build per-core tile streams + shared schedule."""
    es = np.asarray(edge_src).astype(np.int64)
    ed = np.asarray(edge_dst).astype(np.int64)
    core = ed // DST_PER_CORE
    shard = ed % DST_PER_CORE
    blk = shard // BLK
    edl = shard % BLK

    # per (core, blk, half) edge index lists
    counts = np.zeros((NCORES, N_BLK, 2), dtype=np.int64)
    lists = [[[None, None] for _ in range(N_BLK)] for _ in range(NCORES)]
    half = (es >= HALF).astype(np.int64)
    order = np.lexsort((half, blk, core))
    es_s, blk_s, edl_s, core_s, half_s = (
        es[order], blk[order], edl[order], core[order], half[order])
    key = ((core_s * N_BLK) + blk_s) * 2 + half_s
    uniq, starts = np.unique(key, return_index=True)
    starts = list(starts) + [len(key)]
    for i, k in enumerate(uniq):
        c = int(k) // (N_BLK * 2)
        b = (int(k) // 2) % N_BLK
        h = int(k) % 2
        sl = slice(starts[i], starts[i + 1])
        lists[c][b][h] = (es_s[sl], edl_s[sl])
        counts[c, b, h] = starts[i + 1] - starts[i]

    # shared tile schedule: per block, lo tiles then hi tiles (max over cores)
    t_lo = np.maximum(1, np.ceil(counts[:, :, 0] / P).astype(np.int64).max(axis=0))
    t_hi = np.ceil(counts[:, :, 1] / P).astype(np.int64).max(axis=0)
    tiles = []  # (blk, half, first_in_blk, last_in_blk)
    for b in range(N_BLK):
        n = int(t_lo[b] + t_hi[b])
        for j in range(int(t_lo[b])):
            tiles.append((b, 0, j == 0, j == n - 1))
        for j in range(int(t_hi[b])):
            tiles.append((b, 1, t_lo[b] == 0 and j == 0, j == int(t_hi[b]) - 1))
    T = len(tiles)
    while T % BATCH != 0:
        tiles.append((N_BLK - 1, 0, False, False))
        T += 1
    # padded trailing tiles fold into block N_BLK-1's accumulation
    lastb = N_BLK - 1
    idxs = [i for i, t in enumerate(tiles) if t[0] == lastb]
    for j, i in enumerate(idxs):
        b, h, _, _ = tiles[i]
        tiles[i] = (b, h, j == 0, j == len(idxs) - 1)

    NB = T // BATCH

    # per-batch gather runs: (half, off_tiles, ntiles) split at batch bounds
    runs = []
    for bi in range(NB):
        rr = []
        t0 = bi * BATCH
        cur_h, cur_off = tiles[t0][1], 0
        for j in range(1, BATCH):
            h = tiles[t0 + j][1]
            if h != cur_h:
                rr.append((cur_h, cur_off, j - cur_off))
                cur_h, cur_off = h, j
        rr.append((cur_h, cur_off, BATCH - cur_off))
        runs.append(rr)

    # per-core arrays
    per_core = []
    for c in range(NCORES):
        es_tiles = np.zeros((T, P), dtype=np.int64)
        edl_tiles = np.full((T, P), -1.0, dtype=np.float32)
        fill = np.zeros(N_BLK * 2, dtype=np.int64)
        for ti, (b, h, _, _) in enumerate(tiles):
            ent = lists[c][b][h]
            if ent is None:
                if h == 1:
                    es_tiles[ti, :] = HALF
                continue
            e_arr, l_arr = ent
            k = fill[b * 2 + h]
            take = min(P, max(0, len(e_arr) - k))
            if take > 0:
                es_tiles[ti, :take] = e_arr[k:k + take]
                edl_tiles[ti, :take] = l_arr[k:k + take]
            if take < P and h == 1:
                es_tiles[ti, take:] = HALF
            fill[b * 2 + h] = k + take
        loc = es_tiles.copy()
        for ti, (b, h, _, _) in enumerate(tiles):
            if h == 1:
                loc[ti] -= HALF
        esw = np.zeros((NB, P, BATCH * P // 16), dtype=np.int16)
        for bi in range(NB):
            flat = loc[bi * BATCH:(bi + 1) * BATCH].reshape(-1).astype(np.int16)
            esw[bi] = _wrap16(flat)
        el_b = edl_tiles.reshape(NB, BATCH, P).transpose(0, 2, 1)  # [NB,P,8]
        edl1 = edl_tiles.reshape(NB, BATCH * P)                    # [NB,1024]
        per_core.append({
            "esw": esw,
            "el": np.ascontiguousarray(el_b).astype(ml_dtypes.bfloat16),
            "edl1": np.ascontiguousarray(edl1).astype(ml_dtypes.bfloat16),
        })
    return tiles, runs, NB, per_core


def _build(tiles, runs, NB):
    nc = bacc.Bacc("TRN2", target_bir_lowering=False, debug=False,
                   enable_asserts=True, num_devices=NCORES,
                   num_swdge_queues=4)

    xsT = nc.dram_tensor("xsT", [128, SRC_PAD], BF, kind="ExternalInput")
    xdT = nc.dram_tensor("xdT", [128, DST_PAD], F32, kind="ExternalInput")
    wsrc = nc.dram_tensor("wsrc", [128, 128], BF, kind="ExternalInput")
    rhs_d = nc.dram_tensor("rhs_d", [128, 132], F32, kind="ExternalInput")
    b_row = nc.dram_tensor("b_row", [1, 128], F32, kind="ExternalInput")
    att8 = nc.dram_tensor("att8", [1, BATCH * P], BF, kind="ExternalInput")
    esw = nc.dram_tensor("esw", [NB, P, BATCH * P // 16], mybir.dt.int16,
                         kind="ExternalInput")
    el = nc.dram_tensor("el", [NB, P, BATCH], BF, kind="ExternalInput")
    edl1 = nc.dram_tensor("edl1", [NB, BATCH * P], BF, kind="ExternalInput")
    kin = nc.dram_tensor("kin", [1, 1], mybir.dt.int32, kind="ExternalInput")
    y = nc.dram_tensor("y", [DST_PAD, 128], F32, kind="ExternalOutput")

    table_lo = nc.dram_tensor("table_lo", [HALF, ROW], BF)
    table_hi = nc.dram_tensor("table_hi", [SRC_PAD - HALF, ROW], BF)

    with tile.TileContext(nc) as tc:
        with tc.tile_pool(name="const", bufs=1) as cpool, \
             tc.tile_pool(name="resident", bufs=1) as rpool:
            kt = cpool.tile([1, 1], mybir.dt.int32)
            nc.sync.dma_start(out=kt[:], in_=kin[:, :])
            kv = nc.values_load(kt[0:1, 0:1], min_val=0, max_val=100000,
                                skip_runtime_bounds_check=True)
            iota_i = cpool.tile([P, P], mybir.dt.int32)
            nc.gpsimd.iota(iota_i[:], pattern=[[1, P]], base=0,
                           channel_multiplier=0)
            iota_rep = cpool.tile([P, P], BF)
            nc.vector.tensor_copy(iota_rep[:], iota_i[:])
            iotap_i = cpool.tile([P, BATCH * P], mybir.dt.int32)
            nc.gpsimd.iota(iotap_i[:], pattern=[[0, BATCH * P]], base=0,
                           channel_multiplier=1)
            iotap = cpool.tile([P, BATCH * P], BF)
            nc.vector.tensor_copy(iotap[:], iotap_i[:])
            ones1 = cpool.tile([1, P], F32)
            nc.vector.memset(ones1[:], 1.0)
            wsrc_t = cpool.tile([P, 128], BF)
            nc.sync.dma_start(out=wsrc_t[:], in_=wsrc[:, :])
            rhsd_t = cpool.tile([P, 132], F32)
            nc.sync.dma_start(out=rhsd_t[:], in_=rhs_d[:, :])
            b_t = cpool.tile([1, P], F32)
            nc.sync.dma_start(out=b_t[:], in_=b_row[:, :])
            attrep = cpool.tile([P, BATCH * P], BF)
            nc.sync.dma_start(out=attrep[:],
                              in_=att8[0:1, :].to_broadcast([P, BATCH * P]))

            self_all = rpool.tile([P, N_BLK * 128], F32)
            alpha_d = rpool.tile([P, N_BLK * 4], BF)

            with tc.For_i(0, kv) as _i:
                # ---------------- phase 0: src table ----------------
                GRP = 8
                nsrc_blk = SRC_PAD // P                      # 391
                ngrp = math.ceil(nsrc_blk / GRP)
                with tc.tile_pool(name="p1x", bufs=3) as xpool, \
                     tc.tile_pool(name="p1f", bufs=3) as fpool, \
                     tc.tile_pool(name="p1ps", bufs=4, space="PSUM") as psum:
                    for g in range(ngrp):
                        j0 = g * GRP
                        jn = min(GRP, nsrc_blk - j0)
                        xg = xpool.tile([P, GRP * 128], BF, tag="xg")
                        nc.sync.dma_start(
                            out=xg[:, :jn * 128],
                            in_=xsT[:, j0 * 128:(j0 + jn) * 128])
                        fb = fpool.tile([P, GRP * ROW], BF, tag="fb")
                        for j4 in range(jn):
                            jj = j0 + j4
                            ps = psum.tile([P, 128], F32, tag="ps")
                            nc.tensor.matmul(
                                ps[:], lhsT=xg[:, j4 * 128:(j4 + 1) * 128],
                                rhs=wsrc_t[:], start=True, stop=True)
                            dst_sl = fb[:, j4 * ROW:(j4 + 1) * ROW]
                            if jj % 2 == 0:
                                nc.vector.tensor_copy(dst_sl, ps[:])
                            else:
                                nc.scalar.copy(dst_sl, ps[:])
                        r0 = j0 * P
                        fb3 = fb[:].rearrange("p (j c) -> p j c", c=ROW)
                        segs = []
                        if r0 < HALF:
                            nlo = min(jn, (HALF - r0) // P)
                            segs.append((table_lo, r0, 0, nlo))
                            if nlo < jn:
                                segs.append((table_hi, 0, nlo, jn - nlo))
                        else:
                            segs.append((table_hi, r0 - HALF, 0, jn))
                        for (tdst, rr, joff, jcnt) in segs:
                            nc.sync.dma_start(
                                out=tdst[rr:rr + jcnt * P, :].rearrange(
                                    "(j p) c -> p j c", p=P),
                                in_=fb3[:, joff:joff + jcnt, :])

                # ---------------- phase 0: dst side ----------------
                with tc.tile_pool(name="p0", bufs=3) as pool, \
                     tc.tile_pool(name="p0ps", bufs=3, space="PSUM") as psum, \
                     tc.tile_pool(name="p0ps2", bufs=2, space="PSUM") as psum2:
                    DG = 8
                    for g0 in range(0, N_BLK, DG):
                        gn = min(DG, N_BLK - g0)
                        xdg = pool.tile([P, DG * 128], F32, tag="xdg")
                        nc.sync.dma_start(
                            out=xdg[:, :gn * 128],
                            in_=xdT[:, g0 * 128:(g0 + gn) * 128])
                        for jo in range(gn):
                            j = g0 + jo
                            xsl = xdg[:, jo * 128:(jo + 1) * 128]
                            ps_s = psum2.tile([P, P], F32, tag="ps_s")
                            nc.tensor.matmul(ps_s[:], lhsT=ones1[:],
                                             rhs=b_t[:],
                                             start=True, stop=False)
                            nc.tensor.matmul(ps_s[:], lhsT=xsl,
                                             rhs=rhsd_t[:, 0:128],
                                             start=False, stop=True)
                            ps_a = psum.tile([P, 4], F32, tag="ps_a")
                            nc.tensor.matmul(ps_a[:], lhsT=xsl,
                                             rhs=rhsd_t[:, 128:132],
                                             start=True, stop=True)
                            if j % 2 == 0:
                                nc.scalar.copy(
                                    self_all[:, j * 128:(j + 1) * 128],
                                    ps_s[:])
                            else:
                                nc.vector.tensor_copy(
                                    self_all[:, j * 128:(j + 1) * 128],
                                    ps_s[:])
                            nc.vector.tensor_copy(
                                alpha_d[:, j * 4:(j + 1) * 4], ps_a[:])

                # ---------------- edge phase ----------------
                with tc.tile_pool(name="eg", bufs=6) as gpool, \
                     tc.tile_pool(name="ei", bufs=6) as ipool, \
                     tc.tile_pool(name="er", bufs=6) as erpool, \
                     tc.tile_pool(name="es", bufs=6) as spool, \
                     tc.tile_pool(name="ea", bufs=4) as apool, \
                     tc.tile_pool(name="em", bufs=6) as mpool, \
                     tc.tile_pool(name="eu", bufs=6) as upool, \
                     tc.tile_pool(name="eo", bufs=4) as opool, \
                     tc.tile_pool(name="eps", bufs=4, space="PSUM") as psA, \
                     tc.tile_pool(name="eac", bufs=3, space="PSUM") as psB:
                    qn = 0
                    acc = None
                    for bi in range(NB):
                        it = ipool.tile([P, BATCH * P // 16], mybir.dt.int16,
                                        tag="it")
                        nc.sync.dma_start(out=it[:], in_=esw[bi, :, :])
                        elb = ipool.tile([P, BATCH], BF, tag="elb")
                        nc.sync.dma_start(out=elb[:], in_=el[bi, :, :])
                        er1 = erpool.tile([1, BATCH * P], BF, tag="er1")
                        nc.scalar.dma_start(out=er1[:],
                                            in_=edl1[bi:bi + 1, :])
                        er = erpool.tile([P, BATCH * P], BF, tag="er")
                        nc.gpsimd.partition_broadcast(er[:], er1[:],
                                                      channels=P)
                        g8 = gpool.tile([P, BATCH * ROW], BF, tag="g8")
                        g83 = g8[:].rearrange("p (t c) -> p t c", c=ROW)
                        for (h, off, ntl) in runs[bi]:
                            tsrc = table_hi if h == 1 else table_lo
                            nc.gpsimd.dma_gather(
                                out_ap=g83[:, off:off + ntl, :],
                                in_ap=tsrc[:, :],
                                idxs_ap=it[:, off * 8:(off + ntl) * 8],
                                num_idxs=ntl * P,
                                num_idxs_reg=ntl * P,
                                elem_size=ROW,
                                single_packet=False,
                                queue_num=qn % 4,
                            )
                            qn += 1
                        s8 = spool.tile([P, BATCH * P], BF, tag="s8")
                        nc.vector.tensor_tensor(
                            out=s8[:].rearrange("p (t r) -> p t r", r=P),
                            in0=elb[:, :, None].to_broadcast([P, BATCH, P]),
                            in1=iota_rep[:, None, :].to_broadcast(
                                [P, BATCH, P]),
                            op=mybir.AluOpType.is_equal)
                        st8 = spool.tile([P, BATCH * P], BF, tag="st8")
                        nc.vector.tensor_tensor(
                            out=st8[:], in0=iotap[:], in1=er[:],
                            op=mybir.AluOpType.is_equal)
                        amul = apool.tile([P, BATCH * P], BF, tag="amul")
                        nc.vector.tensor_tensor(
                            out=amul[:], in0=g8[:], in1=attrep[:],
                            op=mybir.AluOpType.mult)
                        av4 = upool.tile([P, BATCH * 4], F32, tag="av4")
                        nc.vector.tensor_reduce(
                            av4[:].rearrange("p (g o) -> p g o", o=1),
                            amul[:].rearrange("p (g d) -> p g d", d=D_HEAD),
                            mybir.AxisListType.X, mybir.AluOpType.add)
                        ade = psA.tile([P, BATCH * 4], F32, tag="ade")
                        for t in range(BATCH):
                            blk = tiles[bi * BATCH + t][0]
                            nc.tensor.matmul(
                                ade[:, t * 4:(t + 1) * 4],
                                lhsT=st8[:, t * P:(t + 1) * P],
                                rhs=alpha_d[:, blk * 4:(blk + 1) * 4],
                                start=True, stop=True)
                        u8 = upool.tile([P, BATCH * 4], F32, tag="u8")
                        nc.vector.tensor_tensor(
                            out=u8[:], in0=av4[:], in1=ade[:],
                            op=mybir.AluOpType.add)
                        lk = upool.tile([P, BATCH * 4], F32, tag="lk")
                        nc.vector.scalar_tensor_tensor(
                            out=lk[:], in0=u8[:], scalar=0.2, in1=u8[:],
                            op0=mybir.AluOpType.mult, op1=mybir.AluOpType.max)
                        pexp = mpool.tile([P, BATCH * P], BF, tag="pexp")
                        pexp4 = pexp[:].rearrange(
                            "p (t h d) -> p t h d", h=HEADS, d=D_HEAD)
                        lk3 = lk[:].rearrange("p (t h) -> p t h", h=HEADS)
                        nc.scalar.activation(
                            pexp4, lk3[:, :, :, None].to_broadcast(
                                [P, BATCH, HEADS, D_HEAD]),
                            mybir.ActivationFunctionType.Exp)
                        mp8 = mpool.tile([P, BATCH * 132], BF, tag="mp8")
                        mp83 = mp8[:].rearrange("p (t c) -> p t c", c=132)
                        HT = BATCH // 2
                        nc.vector.tensor_tensor(
                            out=mp83[:, 0:HT, 0:128],
                            in0=g83[:, 0:HT, :],
                            in1=pexp[:].rearrange("p (t c) -> p t c",
                                                  c=P)[:, 0:HT, :],
                            op=mybir.AluOpType.mult)
                        nc.vector.tensor_tensor(
                            out=mp83[:, HT:, 0:128],
                            in0=g83[:, HT:, :],
                            in1=pexp[:].rearrange("p (t c) -> p t c",
                                                  c=P)[:, HT:, :],
                            op=mybir.AluOpType.mult)
                        nc.scalar.copy(
                            mp83[:, :, 128:132],
                            pexp4[:, :, :, 0])
                        for t in range(BATCH):
                            ti = bi * BATCH + t
                            blk, _, first, last = tiles[ti]
                            if first:
                                acc = psB.tile([P, 132], F32, tag="acc")
                            s8t = s8[:, t * P:(t + 1) * P]
                            nc.tensor.matmul(
                                acc[:], lhsT=s8t,
                                rhs=mp83[:, t, :],
                                start=first, stop=last)
                            if last:
                                s1 = upool.tile([P, 4], F32, tag="s1")
                                nc.vector.tensor_scalar_add(
                                    s1[:], acc[:, 128:132], 1e-16)
                                rv = upool.tile([P, 4], F32, tag="rv")
                                nc.vector.reciprocal(rv[:], s1[:])
                                ob = opool.tile([P, P], F32, tag="ob")
                                nc.vector.tensor_tensor(
                                    out=ob[:].rearrange(
                                        "p (h d) -> p h d", h=HEADS),
                                    in0=acc[:, 0:128].rearrange(
                                        "p (h d) -> p h d", h=HEADS),
                                    in1=rv[:, :, None].to_broadcast(
                                        [P, HEADS, D_HEAD]),
                                    op=mybir.AluOpType.mult)
                                ob2 = opool.tile([P, P], F32, tag="ob2")
                                nc.vector.tensor_tensor(
                                    out=ob2[:], in0=ob[:],
                                    in1=self_all[:,
                                                 blk * 128:(blk + 1) * 128],
                                    op=mybir.AluOpType.add)
                                nc.scalar.dma_start(
                                    out=y[blk * P:(blk + 1) * P, :],
                                    in_=ob2[:])
    nc.compile()
    return nc


def _host_arrays(x_src, x_dst, W_src, W_dst, att_src, att_dst, W_self, b_self):
    x_src = np.asarray(x_src, dtype=np.float32)
    x_dst = np.asarray(x_dst, dtype=np.float32)
    W_src = np.asarray(W_src, dtype=np.float32)
    W_dst = np.asarray(W_dst, dtype=np.float32)
    att_src = np.asarray(att_src, dtype=np.float32).reshape(HEADS, D_HEAD)
    att_dst = np.asarray(att_dst, dtype=np.float32).reshape(HEADS, D_HEAD)
    W_self = np.asarray(W_self, dtype=np.float32)
    b_self = np.asarray(b_self, dtype=np.float32)

    A_d = np.zeros((128, HEADS), dtype=np.float32)
    for h in range(HEADS):
        A_d[h * D_HEAD:(h + 1) * D_HEAD, h] = att_dst[h]
    WA_d = W_dst @ A_d                                      # [128, 4]
    rhs_d = np.concatenate([W_self, WA_d], axis=1)          # [128, 132] f32
    x_src_p = np.zeros((SRC_PAD, 128), dtype=np.float32)
    x_src_p[:N_SRC] = x_src
    xsT = np.ascontiguousarray(x_src_p.T).astype(ml_dtypes.bfloat16)
    wsrc_bf = W_src.astype(ml_dtypes.bfloat16)
    att8 = np.tile(att_src.reshape(1, -1), (1, BATCH)).astype(
        ml_dtypes.bfloat16)                                 # [1, 1024]
    return xsT, x_dst, wsrc_bf, rhs_d, b_self.reshape(1, 128), att8


_CACHE = {}


def _get_program(edge_src, edge_dst):
    key = (hash(np.asarray(edge_src).tobytes()),
           hash(np.asarray(edge_dst).tobytes()))
    if key not in _CACHE:
        tiles, runs, NB, per_core = _preprocess(edge_src, edge_dst)
        nc = _build(tiles, runs, NB)
        _CACHE[key] = (nc, per_core)
    return _CACHE[key]


def kernel(x_src, x_dst, edge_src, edge_dst, num_dst,
           W_src, W_dst, att_src, att_dst, W_self, b_self, _k=1):
    nc, per_core = _get_program(edge_src, edge_dst)
    xsT, x_dst_f, wsrc_bf, rhs_d, b_row, att8 = _host_arrays(
        x_src, x_dst, W_src, W_dst, att_src, att_dst, W_self, b_self)

    in_maps = []
    for c in range(NCORES):
        xd = np.zeros((DST_PAD, 128), dtype=np.float32)
        xd[:DST_PER_CORE] = x_dst_f[c * DST_PER_CORE:(c + 1) * DST_PER_CORE]
        in_maps.append({
            "xsT": xsT,
            "xdT": np.ascontiguousarray(xd.T),
            "wsrc": wsrc_bf,
            "rhs_d": rhs_d,
            "b_row": b_row,
            "att8": att8,
            "esw": per_core[c]["esw"],
            "el": per_core[c]["el"],
            "edl1": per_core[c]["edl1"],
            "kin": np.array([[_k]], dtype=np.int32),
        })
    res = run_bass_kernel_spmd(nc, in_maps, list(range(NCORES)))
    out = np.concatenate(
        [res.results[c]["y"][:DST_PER_CORE] for c in range(NCORES)], axis=0)
    return out.astype(np.float32)
